# revision 31
# baseline (speedup 1.0000x reference)
"""ALiBi causal attention on 8 TRN2 NeuronCores (Bass/Tile), v6.

Sharding: core c owns heads (8+c, c) for BOTH batches (head-parallel,
weights column-sharded; a heavy ALiBi head paired with a light one keeps
cores balanced).  Scores are computed transposed (S_T[k, q]) so the ALiBi
k-ramp is a per-partition fp32 bias applied by the ScalarEngine exp, and
the softmax-invariant q-term folds into the score matmul as an extra
contraction row.  P@V runs with V stationary ([128k, 65]) streaming p_t,
so context emerges already transposed ([65d, q]) with the softmax
denominator in row 64 (ones-column of V).  Heads 0..7 use an ALiBi window
(slope*dist/8 > 6 dropped, rel err ~e^-6).  The q axis is processed in two
1024-halves per head so ctx accumulators need only 2 PSUM banks, leaving 6
for triple-buffered score tiles; batch 1's projections are interleaved
into batch 0's attention loop to keep the PE dense while the ScalarE
(the co-critical engine at ~1 elem/lane/cycle @1.2GHz) works through the
exps.  Four AllToAlls (batch x head-slot); Wo's kt 0-3 depend only on the
slot-A collective so its matmul group naturally pipelines across the last
collective.  Compute dtype bf16 (fp32 accumulation in PSUM).
"""

import math

import numpy as np
import ml_dtypes

import bass_rust
import concourse.bass as bass
import concourse.mybir as mybir
import concourse.tile as tile
from concourse.bass_utils import run_bass_kernel_spmd

B, N, D = 2, 2048, 1024
H, HD = 16, 64
NCORES = 8
HPC = H // NCORES      # head slots per core = 2
NT = N // 128          # 16 blocks of 128 along seq
QS = N // 4            # query rows owned per core = 512
QSO = N // NCORES      # query rows owned per core per batch = 256
KT = D // 128          # 8 contraction tiles for d
BF16 = mybir.dt.bfloat16
F32 = mybir.dt.float32
SHIFT = 6.0            # static upper bound of the adjusted logits
NWIN = (16, 7)         # key-block window per head slot (slot A full causal)
NCH = 4                # 512-col q chunks


def _split_multi_waits(nc):
    """This image's walrus rejects >1 sync-wait per instruction; move extra
    waits onto single-wait NoOps spliced just before the instruction in the
    same engine stream (the engine blocks on the NoOps first)."""
    n_split = 0
    for f in nc.m.functions:
        for bb in f.blocks:
            insts = list(bb.instructions)
            new = []
            for inst in insts:
                si = getattr(inst, "sync_info", None)
                waits = list(si.on_wait) if si is not None and si.on_wait else []
                if len(waits) > 1:
                    for idx, w in enumerate(waits[1:]):
                        nop = mybir.InstNoOp(
                            name=f"{inst.name}-xw{idx}", ins=[], outs=[])
                        nop.engine = inst.engine
                        nop.sync_info = bass_rust.SyncInfo(
                            on_wait=[w], on_update=[])
                        new.append(nop)
                    si.on_wait = waits[:1]
                    n_split += 1
                new.append(inst)
            if len(new) != len(insts):
                bb.instructions = new
    return n_split


def _get_slopes(n):
    def pow2(n):
        start = 2 ** (-(2 ** (-(math.log2(n) - 3))))
        return [start * start**i for i in range(n)]

    if math.log2(n).is_integer():
        return pow2(n)
    c = 2 ** math.floor(math.log2(n))
    return pow2(c) + _get_slopes(2 * c)[0::2][: n - c]


def build_nc():
    nc = bass.Bass()

    xT = nc.declare_dram_parameter("xT", [B, 128, KT * N], BF16, isOutput=False)
    wq = nc.declare_dram_parameter("wq", [128, KT * 128], BF16, isOutput=False)
    wk = nc.declare_dram_parameter("wk", [128, KT * 128], BF16, isOutput=False)
    wv = nc.declare_dram_parameter("wv", [128, KT * 128], BF16, isOutput=False)
    wo = nc.declare_dram_parameter("wo", [128, KT * D], BF16, isOutput=False)
    qrow = nc.declare_dram_parameter("qrow", [HPC, N], BF16, isOutput=False)
    kbias = nc.declare_dram_parameter("kbias", [128, HPC * NT], F32, isOutput=False)
    y = nc.declare_dram_parameter("y", [QS, D], F32, isOutput=True)

    # one AllToAll per (batch, head slot): [sender, 64 d-rows, q-slice]
    a2a_in = [[nc.dram_tensor(f"a2a_in{b}{h}", [NCORES, 64, QSO], BF16)
               for h in range(HPC)] for b in range(B)]
    a2a_out = [[nc.dram_tensor(f"a2a_out{b}{h}", [NCORES, 64, QSO], BF16)
                for h in range(HPC)] for b in range(B)]
    groups = [list(range(NCORES))]

    from contextlib import ExitStack

    with tile.TileContext(nc) as tc, ExitStack() as est:
        cpool = est.enter_context(tc.tile_pool(name="const", bufs=1))
        xpool = est.enter_context(tc.tile_pool(name="x", bufs=1))
        qkpool = est.enter_context(tc.tile_pool(name="qk", bufs=1))
        vpool = est.enter_context(tc.tile_pool(name="v", bufs=1))
        ppool = est.enter_context(tc.tile_pool(name="p", bufs=4))
        ctpool = est.enter_context(tc.tile_pool(name="ct", bufs=1))
        cfpool = est.enter_context(tc.tile_pool(name="cf", bufs=1))
        rpool = est.enter_context(tc.tile_pool(name="rc", bufs=2))
        spool = est.enter_context(tc.tile_pool(name="cs", bufs=2))
        opool = est.enter_context(tc.tile_pool(name="ob", bufs=2))
        scps = est.enter_context(tc.tile_pool(name="mm", bufs=3, space="PSUM"))
        ctxps = est.enter_context(tc.tile_pool(name="cx", bufs=1, space="PSUM"))

        # constants
        mask = cpool.tile([128, 128], BF16, tag="mask", name="mask")
        nc.vector.memset(mask[:], 1.0)
        nc.gpsimd.affine_select(
            out=mask[:], in_=mask[:], compare_op=mybir.AluOpType.is_ge,
            fill=0.0, base=0, pattern=[[1, 128]], channel_multiplier=-1,
        )
        ones = cpool.tile([1, 64], BF16, tag="ones", name="ones")
        nc.vector.memset(ones[:], 1.0)
        kb_sb = cpool.tile([128, HPC * NT], F32, tag="kb", name="kb_sb")
        nc.sync.dma_start(out=kb_sb[:], in_=kbias[:])
        wq_sb = cpool.tile([128, KT * 128], BF16, tag="wq", name="wq_sb")
        nc.sync.dma_start(out=wq_sb[:], in_=wq[:])
        wk_sb = cpool.tile([128, KT * 128], BF16, tag="wk", name="wk_sb")
        nc.sync.dma_start(out=wk_sb[:], in_=wk[:])
        wv_sb = cpool.tile([128, KT * 128], BF16, tag="wv", name="wv_sb")
        nc.sync.dma_start(out=wv_sb[:], in_=wv[:])
        wo_sb = cpool.tile([128, KT * D], BF16, tag="wo", name="wo_sb")
        nc.sync.dma_start(out=wo_sb[:], in_=wo[:])

        def fetch_x(b):
            x_t = xpool.tile([128, KT * N], BF16, tag=f"xt{b}", name=f"x_t{b}")
            for kt in range(KT):
                nc.sync.dma_start(
                    out=x_t[:, kt * N:(kt + 1) * N],
                    in_=xT[b][:, kt * N:(kt + 1) * N])
            return x_t

        def alloc_proj(b):
            qe = [qkpool.tile([65, N], BF16, tag=f"qe{b}{h}", name=f"qe{b}{h}")
                  for h in range(HPC)]
            ke = [qkpool.tile([65, N], BF16, tag=f"ke{b}{h}", name=f"ke{b}{h}")
                  for h in range(HPC)]
            v_t = [vpool.tile([128, HPC * 65], BF16, tag=f"v{b}_{nb}",
                              name=f"v{b}_{nb}")
                   for nb in range(NT)]
            return qe, ke, v_t

        def proj_steps(b, x_t, prj):
            """Projection emit-steps (roughly one PSUM group each)."""
            qe, ke, v_t = prj
            steps = []
            for w_sb, dst in ((wq_sb, qe), (wk_sb, ke)):
                for ch in range(NCH):
                    def qk_step(w_sb=w_sb, dst=dst, ch=ch):
                        ps = scps.tile([128, 512], F32, tag="mm", name="ps")
                        for kt in range(KT):
                            nc.tensor.matmul(
                                ps[:],
                                lhsT=w_sb[:, kt * 128:(kt + 1) * 128],
                                rhs=x_t[:, kt * N + ch * 512:
                                        kt * N + (ch + 1) * 512],
                                start=(kt == 0), stop=(kt == KT - 1),
                            )
                        cs = slice(ch * 512, (ch + 1) * 512)
                        nc.vector.tensor_copy(dst[0][0:64, cs], ps[0:64, :])
                        nc.vector.tensor_copy(dst[1][0:64, cs], ps[64:128, :])
                    steps.append(qk_step)
            def rows_step():
                for h in range(HPC):
                    nc.sync.dma_start(out=qe[h][64:65, :], in_=qrow[h:h + 1, :])
                    nc.vector.memset(ke[h][64:65, :], 1.0)
            steps.append(rows_step)
            for nb2 in range(NT // 2):
                def v_step(nb2=nb2):
                    for nb in (2 * nb2, 2 * nb2 + 1):
                        ps = scps.tile([128, 128], F32, tag="mm", name="ps")
                        for kt in range(KT):
                            nc.tensor.matmul(
                                ps[:],
                                lhsT=x_t[:, kt * N + nb * 128:
                                         kt * N + (nb + 1) * 128],
                                rhs=wv_sb[:, kt * 128:(kt + 1) * 128],
                                start=(kt == 0), stop=(kt == KT - 1),
                            )
                        for h in range(HPC):
                            nc.vector.tensor_copy(
                                v_t[nb][:, h * 65: h * 65 + 64],
                                ps[:, h * 64:(h + 1) * 64],
                            )
                            nc.vector.memset(
                                v_t[nb][:, h * 65 + 64: h * 65 + 65], 1.0)
                steps.append(v_step)
            return steps

        def attention(b, prj, filler):
            """Attention for batch b.  filler: emit-steps interleaved into
            the loop (keeps the PE fed while ScalarE exps)."""
            qe, ke, v_t = prj
            n_iters = sum(
                1 for h in range(HPC) for half in range(2)
                for kb in range(NT)
                if kb * 128 < (half + 1) * 1024
                and min(N, (kb + NWIN[h]) * 128) > half * 1024)
            fstate = [0, 0]  # iterations seen, steps emitted

            def fill():
                fstate[0] += 1
                target = fstate[0] * len(filler) // n_iters
                while fstate[1] < min(target, len(filler)):
                    filler[fstate[1]]()
                    fstate[1] += 1

            ct = ctpool.tile([128, N], BF16, tag=f"ct{b}", name=f"ct{b}")
            for h in range(HPC):
                nwin = NWIN[h]
                vs = slice(h * 65, (h + 1) * 65)
                for half in range(2):
                    lo_h, hi_h = half * 1024, (half + 1) * 1024
                    # ctx accumulators for this half's two 512-chunks
                    cx = [ctxps.tile([65, 512], F32, tag=f"cx{i}",
                                     name=f"cx{i}") for i in range(2)]
                    kbs = [kb for kb in range(NT)
                           if kb * 128 < hi_h
                           and min(N, (kb + nwin) * 128) > lo_h]

                    def emit_scores(kb):
                        q0 = kb * 128
                        lo = max(q0, lo_h)
                        hi = min(min(N, (kb + nwin) * 128), hi_h)
                        p_t = ppool.tile([128, 1024], BF16, tag="p",
                                         name="p_t")
                        ps = scps.tile([128, 1024], F32, tag="mm", name="ps")
                        for j in range(lo, hi, 512):
                            jw = min(512, hi - j)
                            nc.tensor.matmul(
                                ps[:, j - lo:j - lo + jw],
                                lhsT=ke[h][:, kb * 128:(kb + 1) * 128],
                                rhs=qe[h][:, j:j + jw],
                            )
                        return (kb, lo, hi, p_t, ps)

                    def emit_exp(st):
                        kb, lo, hi, p_t, ps = st
                        col = h * NT + kb
                        nc.scalar.activation(
                            p_t[:, 0:hi - lo], ps[:, 0:hi - lo],
                            mybir.ActivationFunctionType.Exp,
                            bias=kb_sb[:, col:col + 1], scale=1.0,
                        )

                    def emit_mask(st):
                        # right after the exp: a full iteration of vector-
                        # queue slack before the diagonal P@V needs it
                        kb, lo, hi, p_t, ps = st
                        if kb * 128 >= lo_h:  # diagonal block in this half
                            nc.vector.tensor_tensor(
                                p_t[:, 0:128], p_t[:, 0:128], mask[:],
                                op=mybir.AluOpType.mult,
                            )

                    def emit_pv(st):
                        kb, lo, hi, p_t, ps = st
                        q0 = kb * 128
                        # diagonal (mask-dependent) chunk last: reversed
                        for c in reversed(range(lo // 512,
                                                (hi - 1) // 512 + 1)):
                            clo = max(lo, 512 * c)
                            chi = min(hi, 512 * c + 512)
                            first = max(0, 4 * c - nwin + 1, kbs[0])
                            cxl = cx[c - 2 * half]
                            nc.tensor.matmul(
                                cxl[:, clo - 512 * c:chi - 512 * c],
                                lhsT=v_t[kb][:, vs],
                                rhs=p_t[:, clo - lo:chi - lo],
                                start=(kb == first), stop=(kb == 4 * c + 3),
                                skip_group_check=True,
                            )
                            # last contribution: normalize + stage the chunk
                            if kb == 4 * c + 3:
                                lnd = rpool.tile([1, 512], F32, tag="lnd",
                                                 name="lnd")
                                nc.scalar.activation(
                                    lnd[:], cxl[64:65, :],
                                    mybir.ActivationFunctionType.Ln,
                                )
                                rrow = rpool.tile([1, 512], BF16, tag="rr",
                                                  name="rrow")
                                nc.scalar.activation(
                                    rrow[:], lnd[:],
                                    mybir.ActivationFunctionType.Exp,
                                    scale=-1.0,
                                )
                                bc = scps.tile([64, 512], F32, tag="mm",
                                               name="bc")
                                nc.tensor.matmul(
                                    bc[:], lhsT=ones[0:1, 0:64],
                                    rhs=rrow[0:1, :], start=True, stop=True,
                                )
                                rec = spool.tile([64, 512], F32, tag="cs",
                                                 name="rec")
                                nc.vector.tensor_copy(rec[:], bc[:])
                                nc.vector.tensor_tensor(
                                    ct[h * 64:(h + 1) * 64,
                                       c * 512:(c + 1) * 512],
                                    cxl[0:64, :], rec[:],
                                    op=mybir.AluOpType.mult,
                                )
                                nc.sync.dma_start(
                                    out=a2a_in[b][h][2 * c:2 * c + 2]
                                    .rearrange("j p q -> p j q"),
                                    in_=ct[h * 64:(h + 1) * 64,
                                           c * 512:(c + 1) * 512]
                                    .rearrange("p (j q) -> p j q", j=2),
                                )

                    prev = None
                    for kb in kbs:
                        st = emit_scores(kb)
                        if prev is not None:
                            emit_pv(prev)
                        emit_exp(st)
                        emit_mask(st)
                        fill()
                        prev = st
                    emit_pv(prev)
                # slot h fully staged: kick its collective
                nc.gpsimd.collective_compute(
                    "AllToAll", mybir.AluOpType.bypass, replica_groups=groups,
                    ins=[a2a_in[b][h][:].opt()], outs=[a2a_out[b][h][:].opt()],
                )
            while fstate[1] < len(filler):
                filler[fstate[1]]()
                fstate[1] += 1
            return ct

        def wo_proj(b):
            # cf[kt]: kt 0-3 = slot-A sender pairs, kt 4-7 = slot-B pairs
            cf = []
            for h in range(HPC):
                for k in range(NCORES // 2):
                    t = cfpool.tile([128, QSO], BF16, tag=f"cf{b}_{h}{k}",
                                    name=f"cf{b}_{h}{k}")
                    nc.sync.dma_start(
                        out=t[:],
                        in_=a2a_out[b][h][:]
                        .rearrange("s p q -> (s p) q")[128 * k:128 * (k + 1)])
                    cf.append(t)
            # pass A: slot-A contraction prepaid to SBUF while the slot-B
            # collective is still in flight (no FIFO stall at kt 4)
            part = []
            for q4 in range(QSO // 128):
                for nch in range(D // 512):
                    ps = scps.tile([128, 512], F32, tag="mm", name="wps")
                    for kt in range(KT // 2):
                        nc.tensor.matmul(
                            ps[:],
                            lhsT=cf[kt][:, q4 * 128:(q4 + 1) * 128],
                            rhs=wo_sb[:, kt * D + nch * 512:
                                      kt * D + (nch + 1) * 512],
                            start=(kt == 0), stop=(kt == KT // 2 - 1),
                        )
                    pa = spool.tile([128, 512], F32, tag=f"wp{q4}{nch}",
                                    name=f"wp{q4}{nch}")
                    nc.vector.tensor_copy(pa[:], ps[:])
                    part.append(pa)
            # pass B: slot-B contraction + add the prepaid half
            for q4 in range(QSO // 128):
                ob = opool.tile([128, D], F32, tag="ob", name="ob")
                for nch in range(D // 512):
                    ps = scps.tile([128, 512], F32, tag="mm", name="wps")
                    for kt in range(KT // 2, KT):
                        nc.tensor.matmul(
                            ps[:],
                            lhsT=cf[kt][:, q4 * 128:(q4 + 1) * 128],
                            rhs=wo_sb[:, kt * D + nch * 512:
                                      kt * D + (nch + 1) * 512],
                            start=(kt == KT // 2), stop=(kt == KT - 1),
                        )
                    nc.vector.tensor_tensor(
                        ob[:, nch * 512:(nch + 1) * 512], ps[:],
                        part[2 * q4 + nch][:], op=mybir.AluOpType.add,
                    )
                r0 = b * QSO + q4 * 128
                nc.sync.dma_start(out=y[r0:r0 + 128, :], in_=ob[:])

        # ---- schedule ----
        x0 = fetch_x(0)
        x1 = fetch_x(1)                 # prefetched during batch 0 compute
        prj0 = alloc_proj(0)
        for stp in proj_steps(0, x0, prj0):
            stp()
        prj1 = alloc_proj(1)
        attention(0, prj0, proj_steps(1, x1, prj1))
        attention(1, prj1, [])
        wo_proj(0)                      # overlaps the tail collectives
        wo_proj(1)

    _split_multi_waits(nc)
    return nc


_NC_CACHE = None


def _prep_inputs(x, Wq, Wk, Wv, Wo, bo):
    """Host-side sharding/layout prep. Returns in_maps for the 8 cores."""
    bf = ml_dtypes.bfloat16
    x = np.asarray(x, np.float32)
    slopes = np.array(_get_slopes(H), np.float64)

    # x transposed + tiled: [B, 128, KT*N];  xTr[b, p, kt*N+q] = x[b, q, kt*128+p]
    xTr = np.ascontiguousarray(
        x.transpose(0, 2, 1).reshape(B, KT, 128, N).transpose(0, 2, 1, 3)
        .reshape(B, 128, KT * N)
    ).astype(bf)

    def wtile(w):  # [D, m] -> [128, KT*m]
        m = w.shape[1]
        return np.ascontiguousarray(
            w.reshape(KT, 128, m).transpose(1, 0, 2).reshape(128, KT * m)
        ).astype(bf)

    # Wo rows reordered to the AllToAll arrival layout: slot-A heads of
    # senders 0..7 (= heads 8..15), then slot-B heads (0..7)
    order = list(range(8, 16)) + list(range(0, 8))
    wo_perm = np.asarray(Wo, np.float32).reshape(H, HD, D)[order].reshape(D, D)
    wo_r = wtile(wo_perm)

    q_idx = np.arange(N, dtype=np.float64)
    p = np.arange(128, dtype=np.float64)
    in_maps = []
    for c in range(NCORES):
        heads = (8 + c, c)
        cols = np.r_[np.arange(heads[0] * HD, (heads[0] + 1) * HD),
                     np.arange(heads[1] * HD, (heads[1] + 1) * HD)]
        sl = slopes[list(heads)] / 8.0
        qr = (-sl[:, None] * q_idx[None, :] - SHIFT).astype(bf)
        kb = np.zeros((128, HPC * NT), np.float32)
        for h in range(HPC):
            for t in range(NT):
                kb[:, h * NT + t] = (sl[h] * (t * 128 + p)).astype(np.float32)
        in_maps.append({
            "xT": xTr,
            "wq": wtile(np.asarray(Wq, np.float32)[:, cols] / 8.0),
            "wk": wtile(np.asarray(Wk, np.float32)[:, cols]),
            "wv": wtile(np.asarray(Wv, np.float32)[:, cols]),
            "wo": wo_r,
            "qrow": qr,
            "kbias": kb,
        })
    return in_maps


def _try_device_reset():
    """Best-effort NeuronCore reset via the axon client (clears collective
    state a previously killed run may have left behind)."""
    try:
        import ctypes
        import time as _time

        import jax

        jax.devices()
        lib = ctypes.CDLL("/opt/axon/libaxon_pjrt.so")
        lib.axon_reset.restype = ctypes.c_int64
        lib.axon_reset()
        _time.sleep(5)
    except Exception:
        pass


def kernel(x, Wq, Wk, Wv, Wo, bo):
    global _NC_CACHE
    if _NC_CACHE is None:
        _NC_CACHE = build_nc()
    nc = _NC_CACHE
    in_maps = _prep_inputs(x, Wq, Wk, Wv, Wo, bo)
    try:
        res = run_bass_kernel_spmd(nc, in_maps, list(range(NCORES)))
    except Exception:
        _try_device_reset()
        res = run_bass_kernel_spmd(nc, in_maps, list(range(NCORES)))
    out = np.empty((B, N, D), np.float32)
    for c in range(NCORES):
        for b in range(B):
            out[b, c * QSO:(c + 1) * QSO, :] = \
                res.results[c]["y"][b * QSO:(b + 1) * QSO]
    out += np.asarray(bo, np.float32)[None, None, :]
    return out
